# revision 13
# baseline (speedup 1.0000x reference)
"""Trainium2 Bass kernel for quantized-MoE Bottleneck (nn_Bottleneck_37503654429269).

Strategy
--------
- Data-parallel over batch: 32 samples -> 8 cores x 4 samples.
- Per-sample expert routing is resolved on the host: samples are grouped so
  that each core gets 2 weight-set slots (group0 = 3 samples, group1 = 1
  sample; or 2+2 when all expert counts are even). The host passes each
  slot's expert-quantized integer weights.
- All matmuls run in bf16 at full PE rate: the quantized activations
  (round(clip(.,0,1)*(lv-1)) in [0, lv-1]) and quantized weights
  (round(w/s*n) in [-n, n]) are small integers, exactly representable in
  bf16, so bf16 matmul with fp32 PSUM accumulation is exact.
- Rounding matches jnp.round exactly: adding 2^23 to a nonnegative fp32
  value forces round-to-nearest-even at integer granularity; subtracting
  it back yields round(v). Clamping to [0, lv-1] then reproduces the
  clip+quantize of the reference.
- BN + quantize chains are fused into per-group activation ops with
  per-partition scale/bias vectors (computed on host).
- GroupNorm: per-(channel-tile, sample) sums/sumsq accumulated via
  activation accum_out + tensor_tensor_reduce, partition-reduced with a
  single ones-vector matmul, scalar math on tiny [1, K] tiles, and the
  per-channel scale/bias (P, Q) materialized via K=1 outer-product matmuls.
- Final: out = Relu(S3*P + Q + x + gn_b) via a fused affine_then_add DVE op
  plus a per-partition-bias ReLU on the scalar engine.
"""

import numpy as np

BITS = (2, 4, 8)
EPS = 1e-5
B, C_IN, H, W = 32, 1024, 14, 14
WIDTH, OUTC = 256, 1024
PIX = H * W  # 196
NCORES = 8
RB = np.float32(2.0 ** 23)   # rounding bias

_NC_CACHE = {}


# ----------------------------------------------------------------------------
# Device program
# ----------------------------------------------------------------------------

def _build_nc(group_sizes, stage=99):
    """Build the SPMD Bass program.

    group_sizes: tuple of per-weight-set sample counts, e.g. (3, 1) or (2, 2).
    Sample slots are assigned to groups in order.
    stage: debug cutoff — 1: through conv1, 2: through conv2, 3: through
    conv3+stats-accum, 4: through GN stats math, 99: full kernel.
    """
    from contextlib import ExitStack
    import concourse.bacc as bacc
    import concourse.mybir as mybir
    import concourse.tile as tile

    F32 = mybir.dt.float32
    BF16 = mybir.dt.bfloat16
    ALU = mybir.AluOpType
    ACT = mybir.ActivationFunctionType

    NG = len(group_sizes)
    NS = sum(group_sizes)          # samples per core (4)
    assert NS == 4
    slot0 = [sum(group_sizes[:g]) for g in range(NG)]
    groups = [list(range(slot0[g], slot0[g] + group_sizes[g])) for g in range(NG)]
    # psum column-chunks per group: runs of at most 2 samples (<=392 f32/bank)
    chunks = {g: [groups[g][i:i + 2] for i in range(0, len(groups[g]), 2)]
              for g in range(NG)}

    nc = bacc.Bacc("TRN2", target_bir_lowering=False, debug=False,
                   num_devices=NCORES)

    x_d = nc.dram_tensor("x", [8, 128, 4 * PIX], F32, kind="ExternalInput")
    w1_d = nc.dram_tensor("w1", [NG, 128, 8, 256], BF16, kind="ExternalInput")
    w2_d = nc.dram_tensor("w2", [NG, 128, 9, 2, 256], BF16, kind="ExternalInput")
    w3_d = nc.dram_tensor("w3", [NG, 128, 2, 1024], BF16, kind="ExternalInput")
    xs_d = nc.dram_tensor("xs", [128, NG], F32, kind="ExternalInput")   # lv-1
    xb_d = nc.dram_tensor("xb", [128, NG], F32, kind="ExternalInput")   # lv-1
    a1_d = nc.dram_tensor("a1", [128, 2, NG], F32, kind="ExternalInput")
    b1_d = nc.dram_tensor("b1", [128, 2, NG], F32, kind="ExternalInput")
    a2_d = nc.dram_tensor("a2", [128, 2, NG], F32, kind="ExternalInput")
    b2_d = nc.dram_tensor("b2", [128, 2, NG], F32, kind="ExternalInput")
    gng_d = nc.dram_tensor("gng", [1, 1024], F32, kind="ExternalInput")
    gnb_d = nc.dram_tensor("gnb", [128, 8], F32, kind="ExternalInput")
    # per group g: [c3(s) for (gg,si)] then [c3(s)^2 for (gg,si)], 4*ns each
    cst_d = nc.dram_tensor("cst", [1, sum(8 * n for n in group_sizes)], F32,
                           kind="ExternalInput")
    out_d = nc.dram_tensor("out", [8, 128, 4 * PIX], F32, kind="ExternalOutput")

    with tile.TileContext(nc) as tc, ExitStack() as ctx:
        res = ctx.enter_context(tc.tile_pool(name="res", bufs=1))
        rot = ctx.enter_context(tc.tile_pool(name="rot", bufs=4))
        mmp = ctx.enter_context(tc.tile_pool(name="mmp", bufs=6, space="PSUM"))
        smp = ctx.enter_context(tc.tile_pool(name="smp", bufs=1, space="PSUM"))

        # ---------------- constant / input loads ----------------
        X = []
        for m in range(8):
            xt = res.tile([128, 4 * PIX], F32, name=f"X{m}", tag=f"X{m}")
            nc.sync.dma_start(out=xt, in_=x_d.ap()[m])
            X.append(xt)
        XS = res.tile([128, NG], F32, name="XS", tag="XS")
        nc.sync.dma_start(out=XS, in_=xs_d.ap())
        XB = res.tile([128, NG], F32, name="XB", tag="XB")
        nc.sync.dma_start(out=XB, in_=xb_d.ap())
        A1 = res.tile([128, 2, NG], F32, name="A1", tag="A1")
        nc.sync.dma_start(out=A1, in_=a1_d.ap())
        B1 = res.tile([128, 2, NG], F32, name="B1", tag="B1")
        nc.sync.dma_start(out=B1, in_=b1_d.ap())
        A2 = res.tile([128, 2, NG], F32, name="A2", tag="A2")
        nc.sync.dma_start(out=A2, in_=a2_d.ap())
        B2 = res.tile([128, 2, NG], F32, name="B2", tag="B2")
        nc.sync.dma_start(out=B2, in_=b2_d.ap())
        GNG = res.tile([1, 1024], F32, name="GNG", tag="GNG")
        nc.sync.dma_start(out=GNG, in_=gng_d.ap())
        GNB = res.tile([128, 8], F32, name="GNB", tag="GNB")
        nc.sync.dma_start(out=GNB, in_=gnb_d.ap())
        CST = res.tile([1, sum(8 * n for n in group_sizes)], F32, name="CST",
                       tag="CST")
        nc.sync.dma_start(out=CST, in_=cst_d.ap())
        W1 = []
        W2 = []
        W3 = []
        for g in range(NG):
            w1t = res.tile([128, 8, 256], BF16, name=f"W1_{g}", tag=f"W1_{g}")
            nc.sync.dma_start(out=w1t, in_=w1_d.ap()[g])
            W1.append(w1t)
            w2t = res.tile([128, 9, 2, 256], BF16, name=f"W2_{g}", tag=f"W2_{g}")
            nc.sync.dma_start(out=w2t, in_=w2_d.ap()[g])
            W2.append(w2t)
            w3t = res.tile([128, 2, 1024], BF16, name=f"W3_{g}", tag=f"W3_{g}")
            nc.sync.dma_start(out=w3t, in_=w3_d.ap()[g])
            W3.append(w3t)
        ONES = res.tile([128, 1], F32, name="ONES", tag="ONES")
        nc.vector.memset(ONES, 1.0)

        if stage == 0:
            # loads only + trivial consumer of each input family
            z = res.tile([128, 4 * PIX], F32, name="z0", tag="z0")
            nc.vector.tensor_scalar(out=z, in0=X[0], scalar1=2.0, scalar2=None,
                                    op0=ALU.mult)
            nc.sync.dma_start(out=out_d.ap()[0], in_=z)
            zw = res.tile([128, 256], F32, name="zw", tag="zw")
            nc.vector.tensor_copy(out=zw, in_=W1[0][:, 0, :])
            nc.sync.dma_start(out=out_d.ap()[1][:, 0:256], in_=zw)

        # ---------------- x quantization ----------------
        # Xq[kt][g] = round(clip(x,0,1)*(lv_g-1)) as bf16 integers
        Xq = [[None] * NG for _ in range(8)]
        for kt in range(8 if stage >= 1 else 0):
            cl = rot.tile([128, 4 * PIX], F32, name="cl", tag="cl")
            nc.vector.tensor_scalar(
                out=cl, in0=X[kt], scalar1=0.0, scalar2=1.0,
                op0=ALU.max, op1=ALU.min)
            for g in range(NG):
                ns = group_sizes[g]
                u = rot.tile([128, ns * PIX], F32, name="xu", tag=f"xu{g}")
                nc.vector.tensor_scalar(
                    out=u, in0=cl[:, slot0[g] * PIX:(slot0[g] + ns) * PIX],
                    scalar1=XS[:, g:g + 1], scalar2=float(RB),
                    op0=ALU.mult, op1=ALU.add)
                xq = res.tile([128, ns * PIX], BF16, name=f"Xq{kt}_{g}",
                              tag=f"Xq{kt}_{g}")
                Xq[kt][g] = xq
                nc.vector.tensor_scalar(
                    out=xq, in0=u, scalar1=float(RB), scalar2=None,
                    op0=ALU.subtract)

        # ---------------- conv1 (1x1, 1024->256) + bn1 + quant ------
        # padded conv2 input: HP[mo][g] : [128, ns, 16, 18] bf16, pad = 0
        HP = [[None] * NG for _ in range(2)]
        for mo in range(2 if stage >= 1 else 0):
            for g in range(NG):
                ns = group_sizes[g]
                hp = res.tile([128, ns, 16, 18], BF16, name=f"HP{mo}_{g}",
                              tag=f"HP{mo}_{g}")
                nc.vector.memset(hp, 0.0)
                HP[mo][g] = hp

        def bn_quant_chain(ps, ci_samples, g, mo, A, Bt, out_writer):
            """psum [128, nchunk*PIX] -> bn affine -> round -> clamp -> bf16."""
            nchunk = len(ci_samples)
            tpr = rot.tile([128, nchunk * PIX], F32, name="tpr", tag="tpr")
            nc.scalar.activation(
                out=tpr, in_=ps, func=ACT.Identity,
                bias=Bt[:, mo, g:g + 1], scale=A[:, mo, g:g + 1])
            rr = rot.tile([128, nchunk * PIX], F32, name="rr", tag="rr")
            nc.vector.tensor_scalar(
                out=rr, in0=tpr, scalar1=float(RB), scalar2=float(RB),
                op0=ALU.add, op1=ALU.subtract)
            out_writer(rr)

        for g in range(NG if stage >= 1 else 0):
            for mo in range(2):
                for ch in chunks[g]:
                    nchunk = len(ch)
                    c0 = ch[0] - slot0[g]  # sample index within group
                    ps = mmp.tile([128, nchunk * PIX], F32, name="c1ps",
                                  tag="mm")
                    for kt in range(8):
                        nc.tensor.matmul(
                            ps,
                            W1[g][:, kt, mo * 128:(mo + 1) * 128],
                            Xq[kt][g][:, c0 * PIX:(c0 + nchunk) * PIX],
                            start=(kt == 0), stop=(kt == 7))

                    def w1_out(rr, g=g, mo=mo, c0=c0, nchunk=nchunk):
                        nc.vector.tensor_scalar(
                            out=HP[mo][g][:, c0:c0 + nchunk, 1:15, 2:16],
                            in0=rr.rearrange("p (s y x) -> p s y x",
                                             s=nchunk, y=14),
                            scalar1=0.0, scalar2=XB[:, g:g + 1],
                            op0=ALU.max, op1=ALU.min)
                    bn_quant_chain(ps, ch, g, mo, A1, B1, w1_out)

        # ---------------- conv2 (3x3, 256->256) + bn2 + quant -------
        Q2 = [[None] * NG for _ in range(2)]
        for mo in range(2):
            for g in range(NG):
                ns = group_sizes[g]
                Q2[mo][g] = res.tile([128, ns * PIX], BF16, name=f"Q2{mo}_{g}",
                                     tag=f"Q2{mo}_{g}")
        if stage == 1:
            nc.gpsimd.dma_start(
                out=out_d.ap()[0][:, 0:PIX],
                in_=HP[0][0][:, 0:1, 1:15, 2:16])
        for g in range(NG if stage >= 2 else 0):
            for mo in range(2):
                for ch in chunks[g]:
                    nchunk = len(ch)
                    c0 = ch[0] - slot0[g]
                    ps = mmp.tile([128, nchunk, 14, 14], F32, name="c2ps",
                                  tag="mm")
                    first = True
                    for ti, (dy, dx) in enumerate(
                            (dy, dx) for dy in range(3) for dx in range(3)):
                        for kt in range(2):
                            nc.tensor.matmul(
                                ps,
                                W2[g][:, ti, kt, mo * 128:(mo + 1) * 128],
                                HP[kt][g][:, c0:c0 + nchunk,
                                          dy:dy + 14, dx + 1:dx + 15],
                                start=first, stop=(ti == 8 and kt == 1))
                            first = False

                    def w2_out(rr, g=g, mo=mo, c0=c0, nchunk=nchunk):
                        nc.vector.tensor_scalar(
                            out=Q2[mo][g][:, c0 * PIX:(c0 + nchunk) * PIX],
                            in0=rr,
                            scalar1=0.0, scalar2=XB[:, g:g + 1],
                            op0=ALU.max, op1=ALU.min)
                    bn_quant_chain(ps.rearrange("p s y x -> p (s y x)"),
                                   ch, g, mo, A2, B2, w2_out)

        # ---------------- conv3 (1x1, 256->1024) + GN stats ----------------
        S3 = [[None] * NG for _ in range(8)]
        V = [[None] * NG for _ in range(8)]
        ST = [None] * NG
        for g in range(NG):
            ns = group_sizes[g]
            ST[g] = res.tile([128, 16 * ns], F32, name=f"ST{g}", tag=f"ST{g}")
            for mo in range(8):
                S3[mo][g] = res.tile([128, ns * PIX], F32, name=f"S3_{mo}_{g}",
                                     tag=f"S3_{mo}_{g}")
                V[mo][g] = res.tile([128, ns * PIX], F32, name=f"V{mo}_{g}",
                                    tag=f"V{mo}_{g}")

        PQ = [None] * NG

        if stage == 2:
            nc.gpsimd.dma_start(out=out_d.ap()[0][:, 0:PIX],
                                in_=Q2[0][0][:, 0:PIX])

        for g in range(NG if stage >= 3 else 0):
            ns = group_sizes[g]
            for mo in range(8):
                for ch in chunks[g]:
                    nchunk = len(ch)
                    c0 = ch[0] - slot0[g]
                    ps = mmp.tile([128, nchunk * PIX], F32, name="c3ps",
                                  tag="mm")
                    for kt in range(2):
                        nc.tensor.matmul(
                            ps,
                            W3[g][:, kt, mo * 128:(mo + 1) * 128],
                            Q2[kt][g][:, c0 * PIX:(c0 + nchunk) * PIX],
                            start=(kt == 0), stop=(kt == 1))
                    for ci in range(nchunk):
                        si = c0 + ci
                        s3sl = S3[mo][g][:, si * PIX:(si + 1) * PIX]
                        nc.scalar.activation(
                            out=s3sl, in_=ps[:, ci * PIX:(ci + 1) * PIX],
                            func=ACT.Copy, bias=0.0, scale=1.0,
                            accum_out=ST[g][:, mo * ns + si:mo * ns + si + 1])
                        sqs = rot.tile([128, PIX], F32, name="sqs", tag="sqs")
                        nc.vector.scalar_tensor_tensor(
                            out=sqs, in0=s3sl, scalar=1.0, in1=s3sl,
                            op0=ALU.mult, op1=ALU.mult,
                            accum_out=ST[g][:, (8 + mo) * ns + si:
                                            (8 + mo) * ns + si + 1])

            # ---------- GN statistics for this group ----------
            if stage == 3:
                nc.sync.dma_start(out=out_d.ap()[0][:, g * 48:g * 48 + 16 * ns],
                                  in_=ST[g])
                continue
            nsc = 4 * ns  # number of (gg, si) stat entries
            red = smp.tile([1, 16 * ns], F32, name="red", tag="red")
            nc.tensor.matmul(red, ONES, ST[g], start=True, stop=True)
            Tg = res.tile([1, 16 * ns], F32, name=f"T{g}", tag=f"T{g}")
            nc.scalar.activation(out=Tg, in_=red, func=ACT.Copy,
                                 bias=0.0, scale=1.0)
            # pair-add channel-tile halves: [2(kind),4(gg),2(parity),ns]
            G2 = res.tile([1, 8 * ns], F32, name=f"G2_{g}", tag=f"G2_{g}")
            tv = Tg.rearrange("p (k m o s) -> p k m o s", k=2, m=4, o=2)
            nc.vector.tensor_tensor(
                out=G2.rearrange("p (k m s) -> p k m s", k=2, m=4),
                in0=tv[:, :, :, 0, :], in1=tv[:, :, :, 1, :], op=ALU.add)
            # means: MU2[0:nsc] = mean, MU2[nsc:] = E[x^2]
            MU2 = res.tile([1, 8 * ns], F32, name=f"MU2_{g}", tag=f"MU2_{g}")
            nc.vector.tensor_scalar(out=MU2, in0=G2,
                                    scalar1=1.0 / (2 * 128 * PIX), scalar2=None,
                                    op0=ALU.mult)
            VAR = res.tile([1, 4 * ns], F32, name=f"VAR_{g}", tag=f"VAR_{g}")
            nc.vector.tensor_tensor(out=VAR, in0=MU2[:, 0:nsc],
                                    in1=MU2[:, 0:nsc], op=ALU.mult)
            nc.vector.tensor_tensor(out=VAR, in0=MU2[:, nsc:2 * nsc],
                                    in1=VAR, op=ALU.subtract)
            # y = var*c3^2 + eps ; sd = sqrt(y); rc = 1/sd
            cbase = sum(8 * n for n in group_sizes[:g])
            nc.vector.tensor_tensor(out=VAR, in0=VAR,
                                    in1=CST[:, cbase + nsc:cbase + 2 * nsc],
                                    op=ALU.mult)
            nc.vector.tensor_scalar(out=VAR, in0=VAR, scalar1=EPS, scalar2=None,
                                    op0=ALU.add)
            SD = res.tile([1, 4 * ns], F32, name=f"SD_{g}", tag=f"SD_{g}")
            nc.scalar.activation(out=SD, in_=VAR, func=ACT.Sqrt,
                                 bias=0.0, scale=1.0)
            RC = res.tile([1, 4 * ns], F32, name=f"RC_{g}", tag=f"RC_{g}")
            nc.vector.reciprocal(out=RC, in_=SD)
            # F: [f1 = c3*rc | f2 = -mean*f1]
            Fv = res.tile([1, 8 * ns], F32, name=f"F_{g}", tag=f"F_{g}")
            nc.vector.tensor_tensor(out=Fv[:, 0:nsc], in0=RC,
                                    in1=CST[:, cbase:cbase + nsc], op=ALU.mult)
            nc.vector.scalar_tensor_tensor(
                out=Fv[:, nsc:2 * nsc], in0=MU2[:, 0:nsc], scalar=-1.0,
                in1=Fv[:, 0:nsc], op0=ALU.mult, op1=ALU.mult)
            if stage == 4:
                nc.sync.dma_start(out=out_d.ap()[0][0:1, g * 48:g * 48 + 8 * ns],
                                  in_=Fv)
                continue
            # P,Q outer products: pq[mo] = gn_g[mo-tile] x [f1|f2](gg(mo))
            pqp = smp.tile([128, 8, 2, ns], F32, name="pqp", tag="pqp")
            fvv = Fv.rearrange("p (k m s) -> p k m s", k=2, m=4)
            for mo in range(8):
                nc.tensor.matmul(
                    pqp[:, mo, :, :],
                    GNG[:, mo * 128:(mo + 1) * 128],
                    fvv[:, :, mo // 2, :],
                    start=(mo == 0), stop=(mo == 7), skip_group_check=True)
            PQ[g] = res.tile([128, 8, 2, ns], F32, name=f"PQ{g}", tag=f"PQ{g}")
            nc.scalar.activation(out=PQ[g], in_=pqp, func=ACT.Copy,
                                 bias=0.0, scale=1.0)

            # ---------- final affine + residual + relu + store ----------
            for mo in range(8):
                for si, slot in enumerate(groups[g]):
                    nc.vector.affine_then_add(
                        out=V[mo][g][:, si * PIX:(si + 1) * PIX],
                        in0=S3[mo][g][:, si * PIX:(si + 1) * PIX],
                        in1=X[mo][:, slot * PIX:(slot + 1) * PIX],
                        scale=PQ[g][:, mo, 0, si:si + 1],
                        bias=PQ[g][:, mo, 1, si:si + 1])
                ot = rot.tile([128, ns * PIX], F32, name="ot", tag=f"ot{g}")
                nc.scalar.activation(out=ot, in_=V[mo][g], func=ACT.Relu,
                                     bias=GNB[:, mo:mo + 1], scale=1.0)
                nc.sync.dma_start(
                    out=out_d.ap()[mo][:, slot0[g] * PIX:
                                       (slot0[g] + ns) * PIX],
                    in_=ot)

    nc.compile()
    return nc


# ----------------------------------------------------------------------------
# Host side
# ----------------------------------------------------------------------------

def _quant_w(w, lv):
    """Integer quantization levels k and scale c with k*c == qw(w, lv)."""
    n = max(lv // 2 - 1, 1)
    s = np.float32(np.abs(w).max()) + np.float32(1e-12)
    k = np.round((w.astype(np.float32) / s) * np.float32(n)).astype(np.float32)
    return k, np.float32(s) / np.float32(n)


def _assign_groups(mask):
    """Partition 32 samples into 8 cores x group structure.

    Returns (group_sizes, core_samples, core_experts):
      core_samples[c] = list of 4 sample ids (slot order)
      core_experts[c] = list of NG expert ids (one per group)
    """
    mask = np.asarray(mask).astype(np.int64)
    ids = {e: [int(i) for i in np.nonzero(mask == e)[0]] for e in range(3)}
    counts = [len(ids[e]) for e in range(3)]
    if all(c % 2 == 0 for c in counts):
        group_sizes = (2, 2)
        chunks2 = []
        for e in range(3):
            for j in range(0, counts[e], 2):
                chunks2.append((e, ids[e][j:j + 2]))
        assert len(chunks2) == 16
        core_samples = []
        core_experts = []
        for c in range(8):
            (ea, sa), (eb, sb) = chunks2[2 * c], chunks2[2 * c + 1]
            core_samples.append(sa + sb)
            core_experts.append([ea, eb])
        return group_sizes, core_samples, core_experts

    # (3, 1) grouping: n_e = 3*a_e + b_e, sum(a) = 8, sum(b) = 8
    base = [c % 3 for c in counts]
    need = (8 - sum(base)) // 3
    t = [0, 0, 0]
    for e in range(3):
        cap = (counts[e] - base[e]) // 3
        take = min(cap, need)
        t[e] = take
        need -= take
        if need == 0:
            break
    assert need == 0
    b = [base[e] + 3 * t[e] for e in range(3)]
    a = [(counts[e] - b[e]) // 3 for e in range(3)]
    assert sum(a) == 8 and sum(b) == 8
    trip = []
    single = []
    for e in range(3):
        pos = 0
        for _ in range(a[e]):
            trip.append((e, ids[e][pos:pos + 3]))
            pos += 3
        for _ in range(b[e]):
            single.append((e, [ids[e][pos]]))
            pos += 1
        assert pos == counts[e]
    core_samples = []
    core_experts = []
    for c in range(8):
        ea, sa = trip[c]
        eb, sb = single[c]
        core_samples.append(sa + sb)
        core_experts.append([ea, eb])
    return (3, 1), core_samples, core_experts


def kernel(x, mask, w1, w2, w3, bn1_g, bn1_b, bn1_m, bn1_v,
           bn2_g, bn2_b, bn2_m, bn2_v, gn_g, gn_b):
    import ml_dtypes
    from concourse.bass_utils import run_bass_kernel_spmd

    bf16 = ml_dtypes.bfloat16
    f32 = np.float32
    x = np.asarray(x, f32)
    mask = np.asarray(mask)
    w1 = np.asarray(w1, f32)
    w2 = np.asarray(w2, f32)
    w3 = np.asarray(w3, f32)
    bn1 = [np.asarray(v, f32) for v in (bn1_g, bn1_b, bn1_m, bn1_v)]
    bn2 = [np.asarray(v, f32) for v in (bn2_g, bn2_b, bn2_m, bn2_v)]
    gn_g = np.asarray(gn_g, f32)
    gn_b = np.asarray(gn_b, f32)

    group_sizes, core_samples, core_experts = _assign_groups(mask)
    NG = len(group_sizes)

    # ---- per-expert quantized weights + derived constants
    lv_of = [2 ** b for b in BITS]
    K1, K2, K3 = {}, {}, {}
    CW = {}
    for e in range(3):
        lv = lv_of[e]
        k1, c1 = _quant_w(w1, lv)
        k2, c2 = _quant_w(w2, lv)
        k3, c3 = _quant_w(w3, lv)
        K1[e] = k1.reshape(256, 1024)
        K2[e] = k2.reshape(256, 256, 3, 3)
        K3[e] = k3.reshape(1024, 256)
        CW[e] = (c1, c2, c3)

    inv1 = bn1[0] / np.sqrt(bn1[3] + f32(EPS))
    bb1 = bn1[1] - bn1[2] * inv1
    inv2 = bn2[0] / np.sqrt(bn2[3] + f32(EPS))
    bb2 = bn2[1] - bn2[2] * inv2

    def pack_w(e):
        k1t = K1[e].T.reshape(8, 128, 256).transpose(1, 0, 2)        # [128,8,256]
        k2t = K2[e].transpose(2, 3, 1, 0).reshape(9, 2, 128, 256)    # (tap, kt, p, o)
        k2t = k2t.transpose(2, 0, 1, 3)                              # [128,9,2,256]
        k3t = K3[e].T.reshape(2, 128, 1024).transpose(1, 0, 2)       # [128,2,1024]
        return (k1t.astype(bf16), k2t.astype(bf16), k3t.astype(bf16))

    packed = {e: pack_w(e) for e in set(int(v) for v in np.asarray(mask))}

    in_maps = []
    for c in range(8):
        sids = core_samples[c]
        experts = core_experts[c]

        xc = x[sids].reshape(4, 8, 128, PIX).transpose(1, 2, 0, 3) \
                    .reshape(8, 128, 4 * PIX).copy()

        w1c = np.stack([packed[experts[g]][0] for g in range(NG)])
        w2c = np.stack([packed[experts[g]][1] for g in range(NG)])
        w3c = np.stack([packed[experts[g]][2] for g in range(NG)])

        glv = [lv_of[experts[g]] for g in range(NG)]
        xs = np.broadcast_to(np.array([lv - 1 for lv in glv], f32),
                             (128, NG)).copy()
        xb = xs.copy()

        a1 = np.zeros((128, 2, NG), f32)
        b1 = np.zeros((128, 2, NG), f32)
        a2 = np.zeros((128, 2, NG), f32)
        b2 = np.zeros((128, 2, NG), f32)
        for g in range(NG):
            e = experts[g]
            lv = glv[g]
            c1, c2, c3 = CW[e]
            a1[:, :, g] = (inv1 * c1).reshape(2, 128).T
            b1[:, :, g] = (bb1 * f32(lv - 1)).reshape(2, 128).T
            a2[:, :, g] = (inv2 * c2).reshape(2, 128).T
            b2[:, :, g] = (bb2 * f32(lv - 1)).reshape(2, 128).T

        gng = gn_g.reshape(1, 1024).copy()
        gnb = gn_b.reshape(8, 128).T.copy()

        cst = np.zeros(sum(8 * n for n in group_sizes), f32)
        off = 0
        for g in range(NG):
            ns = group_sizes[g]
            e = experts[g]
            lv = glv[g]
            c3e = CW[e][2] / f32(lv - 1)
            cst[off:off + 4 * ns] = c3e
            cst[off + 4 * ns:off + 8 * ns] = c3e * c3e
            off += 8 * ns
        cst = cst.reshape(1, -1)

        in_maps.append({
            "x": xc, "w1": w1c, "w2": w2c, "w3": w3c,
            "xs": xs, "xb": xb, "a1": a1, "b1": b1, "a2": a2, "b2": b2,
            "gng": gng, "gnb": gnb, "cst": cst,
        })

    import os
    stage = int(os.environ.get("KERNEL_STAGE", "99"))
    key = (group_sizes, stage)
    if key not in _NC_CACHE:
        _NC_CACHE[key] = _build_nc(group_sizes, stage)
    nc = _NC_CACHE[key]

    res = run_bass_kernel_spmd(nc, in_maps, core_ids=list(range(NCORES)))

    out = np.zeros((B, OUTC, H, W), f32)
    for c in range(8):
        oc = res.results[c]["out"]  # [8, 128, 4*PIX]
        oc = oc.reshape(8, 128, 4, PIX).transpose(2, 0, 1, 3) \
               .reshape(4, OUTC, H, W)
        for t, sid in enumerate(core_samples[c]):
            out[sid] = oc[t]
    return out
